# revision 11
# baseline (speedup 1.0000x reference)
"""DeepseekV2 MLA attention (T=2048, H=16) on 8 trn2 cores.

Sharding v2: stage-1 (the low-rank a-projections) is TOKEN-sharded —
core i computes q_c / kv_c / k_pe only for its 256 tokens (8x less
replicated PE work than v1). The normalized kv latent + roped k_pe are
AllGathered (288KB/rank); q_b is computed locally on my tokens for ALL
16 heads (keeps the PE busy during the ~60us collective-subsystem
warmup), and a small AllToAll redistributes q by head. Attention and
o_proj stay head-sharded (2 heads/core); per-core partial outputs
(local heads through o_proj, bf16) are summed on the host.

Device-side layout tricks (kept from v1):
- All attention operands are "transposed" ([feature, t]) so matmul
  contractions land on the partition dim with no PE transposes.
- Scores are S^T[k, q] = K^T q blocks; softmax denominator via a
  ones-vector matmul; no row-max subtraction (scaled scores ~N(0,1));
  normalization applied after P@V.
- RMSNorm r[t]=rsqrt(mean(x^2)+eps) via squares + ones-matmul, applied
  to the stage-1 outputs before the collectives (ln weights folded
  into the b-projections on the host).
- Neox rope folded into duplicated/rotated weight columns so
  rotate-half never crosses partitions.
"""

import numpy as np

T = 2048
HID = 2048
H = 16
NC_ = 8
HLOC = H // NC_          # 2 heads per core
TLOC = T // NC_          # 256 tokens per core
QL = 1536                # q lora
KVL = 512                # kv lora
DN = 128                 # nope dim
DR = 64                  # rope dim
DQK = DN + DR            # 192
DV = 128
EPS = 1e-6
SCALE = float(DQK) ** -0.5
P = 128
QC = 512                 # attention q-chunk
NQC = T // QC
NKB = T // P             # key blocks
NKQ = QL // P            # 12
NKV = KVL // P           # 4
KH = HID // P            # 16 k-steps for stage-1

_CACHE = {}
LAST_RESULTS = None


def _split_multi_waits(nc, mybir):
    """Walrus embeds at most one sem/event wait per TPB instruction; hoist
    extra waits onto preceding same-engine NoOps (queue FIFO keeps order)."""
    n = 0
    for f in nc.m.functions:
        for bb in f.blocks:
            new = []
            for inst in bb.instructions:
                si = getattr(inst, "sync_info", None)
                if si is not None and len(si.on_wait) > 1:
                    waits = list(si.on_wait)
                    for i, wv in enumerate(waits[:-1]):
                        noop = mybir.InstNoOp(
                            name=f"{inst.name}-wsplit{i}",
                            engine=inst.engine,
                            ins=[],
                            outs=[],
                        )
                        noop.bass_nofuse = True
                        noop.sync_info = mybir.SyncInfo(on_wait=[wv], on_update=[])
                        new.append(noop)
                    inst.sync_info = mybir.SyncInfo(
                        on_wait=[waits[-1]], on_update=list(si.on_update)
                    )
                    n += 1
                new.append(inst)
            bb.instructions = new
    return n


def _build_program():
    import concourse.bass as bass
    import concourse.tile as tile
    from concourse import mybir

    f32 = mybir.dt.float32
    bf16 = mybir.dt.bfloat16
    f32r = mybir.dt.float32r
    AF = mybir.ActivationFunctionType

    nc = bass.Bass()
    RG = [list(range(NC_))]

    # ---- per-core external inputs (pre-tiled on host) ----
    hT_d = nc.declare_dram_parameter("hT", [P, KH, TLOC], bf16, isOutput=False)
    wqa_d = nc.declare_dram_parameter("wqa", [P, NKQ, KH, P], bf16, isOutput=False)
    # latent 512 | ropeA dup 128 | ropeB dup 128
    wkva_d = nc.declare_dram_parameter("wkva", [P, NKV + 2, KH, P], bf16, isOutput=False)
    # ALL heads, shard-major: [p, shard, mo(nope_e|nope_o|peA2|peB2), k, j]
    wqb_d = nc.declare_dram_parameter("wqb", [P, NC_, 4, NKQ, P], bf16, isOutput=False)
    wkvbk_d = nc.declare_dram_parameter("wkvbk", [P, NKV, HLOC * DN], bf16, isOutput=False)
    wkvbv_d = nc.declare_dram_parameter("wkvbv", [P, NKV, HLOC * DV], bf16, isOutput=False)
    wo_d = nc.declare_dram_parameter("wo", [P, HLOC, HID], f32r, isOutput=False)
    cosL_d = nc.declare_dram_parameter("cosL", [P, TLOC], f32, isOutput=False)
    sinL_d = nc.declare_dram_parameter("sinL", [P, TLOC], f32, isOutput=False)
    trimask_d = nc.declare_dram_parameter("trimask", [P, P], f32, isOutput=False)
    y_d = nc.declare_dram_parameter("y", [T, HID], bf16, isOutput=True)

    # ---- internal DRAM bounce buffers for the collectives ----
    kv_snd = nc.dram_tensor("kv_snd", [KVL + DR, TLOC], bf16, kind="Internal")
    kv_rcv = nc.dram_tensor(
        "kv_rcv", [NC_, KVL + DR, TLOC], bf16, kind="Internal", addr_space="Shared"
    )
    a_snd = nc.dram_tensor("a_snd", [NC_, 3, P, TLOC], bf16, kind="Internal")
    a_rcv = nc.dram_tensor("a_rcv", [NC_, 3, P, TLOC], bf16, kind="Internal")

    def r32(ap):
        return ap.bitcast(f32r)

    with tile.TileContext(nc) as tc, nc.allow_low_precision(
        reason="bf16/fp32r rounding on PE-matmul operands is intentional"
    ):
        with tc.tile_pool(name="persist", bufs=1) as pp:
            # persistent SBUF tensors (loaded on the scalar DMA queue,
            # which is idle at kernel start; sync carries h + wqa)
            wkvbk_sb = pp.tile([P, NKV, HLOC * DN], bf16, name="wkvbk")
            wkvbv_sb = pp.tile([P, NKV, HLOC * DV], bf16, name="wkvbv")
            trimask_sb = pp.tile([P, P], f32, name="trimask")
            wo_sb = pp.tile([P, HLOC, T], f32r, name="wo")
            cos_sb = pp.tile([P, TLOC], f32, name="cosL")
            nc.scalar.dma_start(out=cos_sb, in_=cosL_d[:, :])
            sin_sb = pp.tile([P, TLOC], f32, name="sinL")
            nc.scalar.dma_start(out=sin_sb, in_=sinL_d[:, :])

            ones_f = pp.tile([P, P], f32, name="ones_f")
            nc.vector.memset(ones_f, 1.0)
            ones_sb = pp.tile([P, 1], f32r, name="ones")
            nc.vector.tensor_copy(ones_sb, ones_f[:, 0:1])
            col_ones = pp.tile([1, P], f32r, name="col_ones")
            nc.vector.tensor_copy(col_ones, ones_f[0:1, :])
            eps_sb = pp.tile([1, 1], f32, name="eps")
            nc.vector.memset(eps_sb, EPS)

            # attention operand tensors (filled after the collectives)
            qTn = [pp.tile([P, T], f32r, name=f"qTn{h}") for h in range(HLOC)]
            qpeT2 = pp.tile([P, T], f32r, name="qpeT2")   # h0 rope rows 0:64, h1 64:128
            KT = [pp.tile([P, T], f32r, name=f"KT{h}") for h in range(HLOC)]
            kpe2 = [pp.tile([P, T], f32, name=f"kpe2{h}") for h in range(HLOC)]
            nc.vector.memset(kpe2[0][DR:P, :], 0.0)
            nc.vector.memset(kpe2[1][0:DR, :], 0.0)
            kvc_g = pp.tile([P, NKV, T], bf16, name="kvc_g")
            V_sb = [pp.tile([P, HLOC * DV], f32r, name=f"v{i}") for i in range(NKB)]

            # ---------------- Stage A: local projections ----------------
            with (
                tc.tile_pool(name="aloc", bufs=1) as ap_,
                tc.tile_pool(name="astream", bufs=2) as sp_,
                tc.tile_pool(name="asmall", bufs=1) as smp,
                tc.tile_pool(name="aps", bufs=2, space="PSUM") as s1ps,
                tc.tile_pool(name="qbps", bufs=4, space="PSUM") as qbps,
                tc.tile_pool(name="ssqps", bufs=1, space="PSUM") as ssqps,
            ):
                h_sb = ap_.tile([P, KH, TLOC], bf16, name="hmy")
                nc.sync.dma_start(out=h_sb, in_=hT_d[:, :, :])

                # ---- kv stage-1 first (feeds the AllGather ASAP) ----
                ssq_kv = ssqps.tile([1, TLOC], f32, name="ssq", bufs=2)
                kvraw = []
                kva_ps = []
                for m in range(NKV + 2):
                    wk_sb = sp_.tile([P, KH, P], bf16, name="wkstream")
                    nc.scalar.dma_start(out=wk_sb, in_=wkva_d[:, m, :, :])
                    ps = s1ps.tile([P, TLOC], f32, name="s1")
                    for k in range(KH):
                        nc.tensor.matmul(
                            ps,
                            lhsT=wk_sb[:, k, :],
                            rhs=h_sb[:, k, :],
                            start=(k == 0),
                            stop=(k == KH - 1),
                        )
                    if m < NKV:
                        kt = ap_.tile([P, TLOC], f32r, name=f"kvraw{m}")
                        nc.vector.tensor_copy(kt, ps)
                        kvraw.append(kt)
                        sq = smp.tile([P, TLOC], f32r, name="sq", bufs=1)
                        nc.scalar.square(sq, ps)
                        nc.tensor.matmul(
                            ssq_kv,
                            lhsT=r32(ones_sb),
                            rhs=r32(sq),
                            start=(m == 0),
                            stop=(m == NKV - 1),
                        )
                    else:
                        kva_ps.append(ps)   # rope A2/B2 read straight from PSUM

                # rkv = rsqrt(mean+eps), broadcast via ones-matmul
                rkv = smp.tile([1, TLOC], f32r, name="rkv")
                nc.scalar.activation(
                    rkv, ssq_kv, func=AF.Sqrt, bias=eps_sb, scale=1.0 / KVL
                )
                nc.vector.reciprocal(rkv, rkv)
                rkvb_ps = s1ps.tile([P, TLOC], f32, name="s1")
                nc.tensor.matmul(rkvb_ps, lhsT=col_ones, rhs=rkv, start=True, stop=True)
                rkv_b = smp.tile([P, TLOC], f32, name="rkvb")
                nc.vector.tensor_copy(rkv_b, rkvb_ps)

                # k_pe rope: (A*cos + B*sin); rows 0:64 are the head-shared k_pe
                t1 = smp.tile([P, TLOC], f32, name="ropet1")
                t2 = smp.tile([P, TLOC], f32, name="ropet2")
                nc.vector.tensor_mul(t1, kva_ps[0], cos_sb)
                nc.vector.tensor_mul(t2, kva_ps[1], sin_sb)
                nc.vector.tensor_add(t1, t1, t2)
                kpe_bf = smp.tile([DR, TLOC], bf16, name="kpebf")
                nc.vector.tensor_copy(kpe_bf, t1[0:DR, :])
                nc.scalar.dma_start(out=kv_snd[KVL : KVL + DR, :], in_=kpe_bf)

                # normalize latent, send
                for m in range(NKV):
                    kvn = ap_.tile([P, TLOC], bf16, name=f"kvn{m}")
                    nc.vector.tensor_mul(kvn, kvraw[m], rkv_b)
                    nc.scalar.dma_start(out=kv_snd[m * P : (m + 1) * P, :], in_=kvn)
                nc.gpsimd.collective_compute(
                    "AllGather",
                    mybir.AluOpType.bypass,
                    replica_groups=RG,
                    ins=[kv_snd[:, :]],
                    outs=[kv_rcv[:, :, :]],
                )
                # small weights needed from the kv_b phase on (~90us in)
                nc.scalar.dma_start(out=wkvbk_sb, in_=wkvbk_d[:, :, :])
                nc.scalar.dma_start(out=wkvbv_sb, in_=wkvbv_d[:, :, :])
                nc.scalar.dma_start(out=trimask_sb, in_=trimask_d[:, :])

                # ---- q stage-1 ----
                ssq_q = ssqps.tile([1, TLOC], f32, name="ssq", bufs=2)
                qcraw = []
                for m in range(NKQ):
                    wq_sb = sp_.tile([P, KH, P], bf16, name="wqstream")
                    nc.sync.dma_start(out=wq_sb, in_=wqa_d[:, m, :, :])
                    ps = s1ps.tile([P, TLOC], f32, name="s1")
                    for k in range(KH):
                        nc.tensor.matmul(
                            ps,
                            lhsT=wq_sb[:, k, :],
                            rhs=h_sb[:, k, :],
                            start=(k == 0),
                            stop=(k == KH - 1),
                        )
                    qt = ap_.tile([P, TLOC], f32r, name=f"qcraw{m}")
                    nc.vector.tensor_copy(qt, ps)
                    qcraw.append(qt)
                    sq = smp.tile([P, TLOC], f32r, name="sq", bufs=1)
                    nc.scalar.square(sq, ps)
                    nc.tensor.matmul(
                        ssq_q,
                        lhsT=r32(ones_sb),
                        rhs=r32(sq),
                        start=(m == 0),
                        stop=(m == NKQ - 1),
                    )

                rq = smp.tile([1, TLOC], f32r, name="rq")
                nc.scalar.activation(
                    rq, ssq_q, func=AF.Sqrt, bias=eps_sb, scale=1.0 / QL
                )
                nc.vector.reciprocal(rq, rq)
                rqb_ps = s1ps.tile([P, TLOC], f32, name="s1")
                nc.tensor.matmul(rqb_ps, lhsT=col_ones, rhs=rq, start=True, stop=True)
                rq_b = smp.tile([P, TLOC], f32, name="rqb")
                nc.vector.tensor_copy(rq_b, rqb_ps)
                qcn = []
                for m in range(NKQ):
                    qn = ap_.tile([P, TLOC], bf16, name=f"qcn{m}")
                    nc.vector.tensor_mul(qn, qcraw[m], rq_b)
                    qcn.append(qn)

                # ---- q_b for ALL heads on my tokens; pack A2A shards ----
                for sh in range(NC_):
                    wqb_sb = sp_.tile([P, 4, NKQ, P], bf16, name="wqbstream")
                    eng = nc.sync if sh % 2 == 0 else nc.scalar
                    eng.dma_start(out=wqb_sb, in_=wqb_d[:, sh, :, :, :])
                    ups = []
                    for mo in range(4):
                        pool = qbps if mo < 2 else s1ps
                        nm = "up" if mo < 2 else "s1"
                        ps = pool.tile([P, TLOC], f32, name=nm)
                        for k in range(NKQ):
                            nc.tensor.matmul(
                                ps,
                                lhsT=wqb_sb[:, mo, k, :],
                                rhs=qcn[k],
                                start=(k == 0),
                                stop=(k == NKQ - 1),
                            )
                        ups.append(ps)
                    for u in range(2):
                        st = smp.tile([P, TLOC], bf16, name="stn", bufs=2)
                        nc.vector.tensor_copy(st, ups[u])
                        nc.sync.dma_start(out=a_snd[sh, u, :, :], in_=st)
                    t3 = smp.tile([P, TLOC], f32, name="ropeq1", bufs=2)
                    t4 = smp.tile([P, TLOC], f32, name="ropeq2", bufs=2)
                    nc.vector.tensor_mul(t3, ups[2], cos_sb)
                    nc.vector.tensor_mul(t4, ups[3], sin_sb)
                    stp = smp.tile([P, TLOC], bf16, name="stp", bufs=2)
                    nc.vector.tensor_add(stp, t3, t4)
                    nc.sync.dma_start(out=a_snd[sh, 2, :, :], in_=stp)
                nc.gpsimd.collective_compute(
                    "AllToAll",
                    mybir.AluOpType.bypass,
                    replica_groups=RG,
                    ins=[a_snd[:, :, :, :]],
                    outs=[a_rcv[:, :, :, :]],
                )

                # ---- consume the gathered kv: kv_b up-projection ----
                for k in range(NKV):
                    nc.scalar.dma_start(
                        out=kvc_g[:, k, :].rearrange("p (r t) -> p r t", t=TLOC),
                        in_=kv_rcv[:, k * P : (k + 1) * P, :].rearrange(
                            "r p t -> p r t"
                        ),
                    )
                # k_pe duplicated into both head slots (cast bf16->f32)
                nc.gpsimd.dma_start(
                    out=kpe2[0][0:DR, :].rearrange("p (r t) -> p r t", t=TLOC),
                    in_=kv_rcv[:, KVL:, :].rearrange("r p t -> p r t"),
                )
                nc.gpsimd.dma_start(
                    out=kpe2[1][DR:P, :].rearrange("p (r t) -> p r t", t=TLOC),
                    in_=kv_rcv[:, KVL:, :].rearrange("r p t -> p r t"),
                )
                # K^T per head over full T
                for mo in range(HLOC):
                    for cc in range(T // TLOC):
                        ps = qbps.tile([P, TLOC], f32, name="up")
                        for k in range(NKV):
                            nc.tensor.matmul(
                                ps,
                                lhsT=wkvbk_sb[:, k, mo * P : (mo + 1) * P],
                                rhs=kvc_g[:, k, cc * TLOC : (cc + 1) * TLOC],
                                start=(k == 0),
                                stop=(k == NKV - 1),
                            )
                        nc.vector.tensor_copy(KT[mo][:, cc * TLOC : (cc + 1) * TLOC], ps)
                # V in natural orientation [t, 2*dv]
                for tt in range(NKB):
                    ps = qbps.tile([P, HLOC * DV], f32, name="up")
                    for k in range(NKV):
                        nc.tensor.matmul(
                            ps,
                            lhsT=kvc_g[:, k, tt * P : (tt + 1) * P],
                            rhs=wkvbv_sb[:, k, :],
                            start=(k == 0),
                            stop=(k == NKV - 1),
                        )
                    nc.vector.tensor_copy(V_sb[tt], ps)

                # ---- consume the A2A'd q (cast bf16->f32r) ----
                for hh in range(HLOC):
                    nc.gpsimd.dma_start(
                        out=qTn[hh][:, :].rearrange("p (r t) -> p r t", t=TLOC),
                        in_=a_rcv[:, hh, :, :].rearrange("r p t -> p r t"),
                    )
                nc.gpsimd.dma_start(
                    out=qpeT2[:, :].rearrange("p (r t) -> p r t", t=TLOC),
                    in_=a_rcv[:, 2, :, :].rearrange("r p t -> p r t"),
                )
                # o_proj weights, needed only at the tail (gpsimd is idle here)
                nc.gpsimd.dma_start(out=wo_sb, in_=wo_d[:, :, :])

            # ---------------- Stage B: attention ----------------
            with (
                tc.tile_pool(name="bpt", bufs=4) as ptp,
                tc.tile_pool(name="bsmall", bufs=3) as bsm,
                tc.tile_pool(name="sps", bufs=2, space="PSUM") as spsp,
                tc.tile_pool(name="otps", bufs=2, space="PSUM") as otpsp,
                tc.tile_pool(name="lps", bufs=2, space="PSUM") as lpsp,
            ):
                OT_sb = [
                    [ptp.tile([P, QC], f32r, name=f"ot{h}_{j}", bufs=1) for j in range(NQC)]
                    for h in range(HLOC)
                ]

                def flush_norm(pend):
                    p_ot, p_l, p_h, p_j = pend
                    recl = bsm.tile([1, QC], f32r, name="recl")
                    nc.vector.reciprocal(recl, p_l)
                    lb_ps = spsp.tile([P, 2 * QC], f32, name="sps2")[:, :QC]
                    nc.tensor.matmul(lb_ps, lhsT=col_ones, rhs=recl, start=True, stop=True)
                    lb = bsm.tile([P, QC], f32, name="lb")
                    nc.scalar.copy(lb, lb_ps)
                    nc.vector.tensor_mul(OT_sb[p_h][p_j], p_ot, lb)

                pend = None
                for h in range(HLOC):
                    for j in range(NQC):
                        ot_ps = otpsp.tile([P, QC], f32, name="otps")
                        pacc = bsm.tile([P, QC], f32r, name="pacc", bufs=2)
                        nkb = 4 * (j + 1)
                        qcol0 = j * QC
                        for kp in range(0, nkb, 2):
                            # two k-blocks share one PSUM pair and ONE wide exp;
                            # diagonal blocks are column-clipped everywhere
                            s2 = spsp.tile([P, 2 * QC], f32, name="sps2")
                            for u in range(2):
                                ki = kp + u
                                diag = (ki // 4 == j)
                                cs = (ki % 4) * P if diag else 0
                                nc.tensor.matmul(
                                    s2[:, u * QC + cs : (u + 1) * QC],
                                    lhsT=r32(KT[h][:, ki * P : (ki + 1) * P]),
                                    rhs=r32(qTn[h][:, qcol0 + cs : qcol0 + QC]),
                                    start=True,
                                    stop=False,
                                )
                                nc.tensor.matmul(
                                    s2[:, u * QC + cs : (u + 1) * QC],
                                    lhsT=r32(kpe2[h][:, ki * P : (ki + 1) * P]),
                                    rhs=r32(qpeT2[:, qcol0 + cs : qcol0 + QC]),
                                    start=False,
                                    stop=True,
                                )
                            pt = ptp.tile([P, 2 * QC], f32r, name="pt")
                            nc.scalar.activation(pt, s2, func=AF.Exp, scale=SCALE)
                            for u in range(2):
                                ki = kp + u
                                diag = (ki // 4 == j)
                                cs = (ki % 4) * P if diag else 0
                                if diag:
                                    nc.gpsimd.tensor_mul(
                                        pt[:, u * QC + cs : u * QC + cs + P],
                                        pt[:, u * QC + cs : u * QC + cs + P],
                                        trimask_sb,
                                    )
                                nc.tensor.matmul(
                                    ot_ps[:, cs:],
                                    lhsT=r32(V_sb[ki][:, h * DV : (h + 1) * DV]),
                                    rhs=r32(pt[:, u * QC + cs : (u + 1) * QC]),
                                    start=(ki == 0),
                                    stop=(ki == nkb - 1),
                                )
                                # probs accumulated on the (idle) gpsimd engine;
                                # ONE denominator matmul per (h, j) afterwards
                                if ki == 0:
                                    nc.gpsimd.tensor_copy(
                                        pacc, pt[:, u * QC : (u + 1) * QC]
                                    )
                                else:
                                    nc.gpsimd.tensor_add(
                                        pacc[:, cs:],
                                        pacc[:, cs:],
                                        pt[:, u * QC + cs : (u + 1) * QC],
                                    )
                            if kp == 2 and pend is not None:
                                flush_norm(pend)
                                pend = None
                        l_ps = lpsp.tile([1, QC], f32, name="lps")
                        nc.tensor.matmul(
                            l_ps, lhsT=r32(ones_sb), rhs=r32(pacc), start=True, stop=True
                        )
                        pend = (ot_ps, l_ps, h, j)
                flush_norm(pend)

                # ---------------- o_proj (partial y, summed on host) ----------------
                for tt in range(T // P):
                    j, sub = tt // 4, (tt % 4) * P
                    for n in range(HID // QC):
                        y_ps = spsp.tile([P, 2 * QC], f32, name="sps2")[:, :QC]
                        for h in range(HLOC):
                            nc.tensor.matmul(
                                y_ps,
                                lhsT=r32(OT_sb[h][j][:, sub : sub + P]),
                                rhs=r32(wo_sb[:, h, n * QC : (n + 1) * QC]),
                                start=(h == 0),
                                stop=(h == HLOC - 1),
                            )
                        y_sb = ptp.tile([P, QC], bf16, name="ysb")
                        nc.scalar.copy(y_sb, y_ps)
                        eng = nc.sync if tt % 2 == 0 else nc.scalar
                        eng.dma_start(
                            out=y_d[tt * P : (tt + 1) * P, n * QC : (n + 1) * QC],
                            in_=y_sb,
                        )
    _split_multi_waits(nc, mybir)
    return nc


def _host_prep(inputs):
    import ml_dtypes

    bf = ml_dtypes.bfloat16
    hs = np.ascontiguousarray(np.asarray(inputs["hidden_states"], np.float32))
    pos = np.asarray(inputs["positions"], np.int32)
    w_qa = np.asarray(inputs["w_qa"], np.float32)
    q_ln = np.asarray(inputs["q_a_ln_w"], np.float32)
    w_qb = np.asarray(inputs["w_qb"], np.float32)
    w_kva = np.asarray(inputs["w_kva"], np.float32)
    kv_ln = np.asarray(inputs["kv_a_ln_w"], np.float32)
    w_kvb = np.asarray(inputs["w_kvb"], np.float32)
    w_o = np.asarray(inputs["w_o"], np.float32)

    # replicated stage-1 weights, baseline layout
    wqa_b = np.ascontiguousarray(
        w_qa.reshape(HID // P, P, QL // P, P).transpose(1, 2, 0, 3)
    ).astype(bf)

    # rope tables (neox), rows duplicated for the 2-head-slot packing
    inv_freq = (1.0 / (10000.0 ** (np.arange(0, DR, 2, dtype=np.float32) / DR))).astype(
        np.float32
    )
    freqs = pos.astype(np.float32)[:, None] * inv_freq[None, :]
    emb = np.concatenate([freqs, freqs], axis=-1)  # [T, 64]
    cosT = np.ascontiguousarray(np.cos(emb).T.astype(np.float32))  # [64, T]
    sinT = np.ascontiguousarray(np.sin(emb).T.astype(np.float32))
    cos2 = np.ascontiguousarray(np.concatenate([cosT, cosT], axis=0))  # [128, T]
    sin2 = np.ascontiguousarray(np.concatenate([sinT, sinT], axis=0))

    def rot_cols(A):
        return np.concatenate([-A[:, DR // 2 :], A[:, : DR // 2]], axis=1)

    # kv a-projection augmented with duplicated rope A/B columns
    kva_lat = w_kva[:, :KVL]
    kva_rope = w_kva[:, KVL:]                       # [2048, 64]
    kva_ropeB = rot_cols(kva_rope)
    wkva_aug = np.concatenate(
        [kva_lat, kva_rope, kva_rope, kva_ropeB, kva_ropeB], axis=1
    )  # [2048, 768]
    wkva_b = np.ascontiguousarray(
        wkva_aug.reshape(HID // P, P, NKV + 2, P).transpose(1, 2, 0, 3)
    ).astype(bf)

    w_qb_f = (w_qb * q_ln[:, None]).reshape(QL, H, DQK)
    w_kvb_f = (w_kvb * kv_ln[:, None]).reshape(KVL, H, DN + DV)
    w_o_r = w_o.reshape(H, DV, HID)

    # q_b for ALL heads, shard-major: [p, shard, mo, k, j]
    shards = []
    for sh in range(NC_):
        e, o = 2 * sh, 2 * sh + 1
        nope_e = w_qb_f[:, e, :DN]
        nope_o = w_qb_f[:, o, :DN]
        peA2 = np.concatenate([w_qb_f[:, e, DN:], w_qb_f[:, o, DN:]], axis=1)
        peB2 = np.concatenate(
            [rot_cols(w_qb_f[:, e, DN:]), rot_cols(w_qb_f[:, o, DN:])], axis=1
        )
        shards.append(np.concatenate([nope_e, nope_o, peA2, peB2], axis=1))  # [QL,512]
    wqb_all = np.stack(shards, axis=1)  # [QL, 8, 512]
    wqb_dev = np.ascontiguousarray(
        wqb_all.reshape(NKQ, P, NC_, 4, P).transpose(1, 2, 3, 0, 4)
    ).astype(bf)

    trimask = np.triu(np.ones((P, P), dtype=np.float32))  # [k, q]: 1 iff q>=k

    per_core = []
    for i in range(NC_):
        hh = [HLOC * i + x for x in range(HLOC)]
        t0 = i * TLOC
        hT = np.ascontiguousarray(
            hs[t0 : t0 + TLOC].T.reshape(KH, P, TLOC).transpose(1, 0, 2)
        ).astype(bf)
        wkvbk = np.ascontiguousarray(
            np.concatenate([w_kvb_f[:, h, :DN] for h in hh], axis=1)
            .reshape(NKV, P, HLOC * DN)
            .transpose(1, 0, 2)
        ).astype(bf)
        wkvbv = np.ascontiguousarray(
            np.concatenate([w_kvb_f[:, h, DN:] for h in hh], axis=1)
            .reshape(NKV, P, HLOC * DV)
            .transpose(1, 0, 2)
        ).astype(bf)
        wo_i = np.ascontiguousarray(
            np.stack([w_o_r[h] for h in hh], axis=0).transpose(1, 0, 2)
        )  # [p, h, HID]
        per_core.append(
            dict(
                hT=hT,
                wqa=wqa_b,
                wkva=wkva_b,
                wqb=wqb_dev,
                wkvbk=wkvbk,
                wkvbv=wkvbv,
                wo=wo_i,
                cosL=np.ascontiguousarray(cos2[:, t0 : t0 + TLOC]),
                sinL=np.ascontiguousarray(sin2[:, t0 : t0 + TLOC]),
                trimask=trimask,
            )
        )
    return per_core


def kernel(**inputs):
    global LAST_RESULTS
    from concourse.bass_utils import run_bass_kernel_spmd

    if "nc" not in _CACHE:
        _CACHE["nc"] = _build_program()
    nc = _CACHE["nc"]

    in_maps = _host_prep(inputs)
    res = run_bass_kernel_spmd(nc, in_maps, core_ids=list(range(NC_)))
    LAST_RESULTS = res
    out = np.zeros((T, HID), dtype=np.float32)
    for r in res.results:
        out += np.asarray(r["y"], dtype=np.float32)
    return out


# revision 13
# speedup vs baseline: 1.0509x; 1.0509x over previous
"""DeepseekV2 MLA attention (T=2048, H=16) on 8 trn2 cores.

Sharding v2: stage-1 (the low-rank a-projections) is TOKEN-sharded —
core i computes q_c / kv_c / k_pe only for its 256 tokens (8x less
replicated PE work than v1). The normalized kv latent + roped k_pe are
AllGathered (288KB/rank); q_b is computed locally on my tokens for ALL
16 heads (keeps the PE busy during the ~60us collective-subsystem
warmup), and a small AllToAll redistributes q by head. Attention and
o_proj stay head-sharded (2 heads/core); per-core partial outputs
(local heads through o_proj, bf16) are summed on the host.

Device-side layout tricks (kept from v1):
- All attention operands are "transposed" ([feature, t]) so matmul
  contractions land on the partition dim with no PE transposes.
- Scores are S^T[k, q] = K^T q blocks; softmax denominator via a
  ones-vector matmul; no row-max subtraction (scaled scores ~N(0,1));
  normalization applied after P@V.
- RMSNorm r[t]=rsqrt(mean(x^2)+eps) via squares + ones-matmul, applied
  to the stage-1 outputs before the collectives (ln weights folded
  into the b-projections on the host).
- Neox rope folded into duplicated/rotated weight columns so
  rotate-half never crosses partitions.
"""

import numpy as np

T = 2048
HID = 2048
H = 16
NC_ = 8
HLOC = H // NC_          # 2 heads per core
TLOC = T // NC_          # 256 tokens per core
QL = 1536                # q lora
KVL = 512                # kv lora
DN = 128                 # nope dim
DR = 64                  # rope dim
DQK = DN + DR            # 192
DV = 128
EPS = 1e-6
SCALE = float(DQK) ** -0.5
P = 128
QC = 512                 # attention q-chunk
NQC = T // QC
NKB = T // P             # key blocks
NKQ = QL // P            # 12
NKV = KVL // P           # 4
KH = HID // P            # 16 k-steps for stage-1

_CACHE = {}
LAST_RESULTS = None


def _split_multi_waits(nc, mybir):
    """Walrus embeds at most one sem/event wait per TPB instruction; hoist
    extra waits onto preceding same-engine NoOps (queue FIFO keeps order)."""
    n = 0
    for f in nc.m.functions:
        for bb in f.blocks:
            new = []
            for inst in bb.instructions:
                si = getattr(inst, "sync_info", None)
                if si is not None and len(si.on_wait) > 1:
                    waits = list(si.on_wait)
                    for i, wv in enumerate(waits[:-1]):
                        noop = mybir.InstNoOp(
                            name=f"{inst.name}-wsplit{i}",
                            engine=inst.engine,
                            ins=[],
                            outs=[],
                        )
                        noop.bass_nofuse = True
                        noop.sync_info = mybir.SyncInfo(on_wait=[wv], on_update=[])
                        new.append(noop)
                    inst.sync_info = mybir.SyncInfo(
                        on_wait=[waits[-1]], on_update=list(si.on_update)
                    )
                    n += 1
                new.append(inst)
            bb.instructions = new
    return n


def _build_program():
    import concourse.bass as bass
    import concourse.tile as tile
    from concourse import mybir

    f32 = mybir.dt.float32
    bf16 = mybir.dt.bfloat16
    f32r = mybir.dt.float32r
    AF = mybir.ActivationFunctionType

    nc = bass.Bass()
    RG = [list(range(NC_))]

    # ---- per-core external inputs (pre-tiled on host) ----
    hT_d = nc.declare_dram_parameter("hT", [P, KH, TLOC], bf16, isOutput=False)
    wqa_d = nc.declare_dram_parameter("wqa", [P, NKQ, KH, P], bf16, isOutput=False)
    # latent 512 | ropeA dup 128 | ropeB dup 128
    wkva_d = nc.declare_dram_parameter("wkva", [P, NKV + 2, KH, P], bf16, isOutput=False)
    # ALL heads, shard-major: [p, shard, mo(nope_e|nope_o|peA2|peB2), k, j]
    wqb_d = nc.declare_dram_parameter("wqb", [P, NC_, 4, NKQ, P], bf16, isOutput=False)
    wkvbk_d = nc.declare_dram_parameter("wkvbk", [P, NKV, HLOC * DN], bf16, isOutput=False)
    wkvbv_d = nc.declare_dram_parameter("wkvbv", [P, NKV, HLOC * DV], bf16, isOutput=False)
    wo_d = nc.declare_dram_parameter("wo", [P, HLOC, HID], f32r, isOutput=False)
    cosL_d = nc.declare_dram_parameter("cosL", [P, TLOC], f32, isOutput=False)
    sinL_d = nc.declare_dram_parameter("sinL", [P, TLOC], f32, isOutput=False)
    trimask_d = nc.declare_dram_parameter("trimask", [P, P], f32, isOutput=False)
    y_d = nc.declare_dram_parameter("y", [T, HID], bf16, isOutput=True)

    # ---- internal DRAM bounce buffers for the collectives ----
    kv_snd = nc.dram_tensor("kv_snd", [KVL + DR, TLOC], bf16, kind="Internal")
    kv_rcv = nc.dram_tensor(
        "kv_rcv", [NC_, KVL + DR, TLOC], bf16, kind="Internal", addr_space="Shared"
    )
    a_snd = nc.dram_tensor("a_snd", [NC_, 3, P, TLOC], bf16, kind="Internal")
    a_rcv = nc.dram_tensor("a_rcv", [NC_, 3, P, TLOC], bf16, kind="Internal")

    def r32(ap):
        return ap.bitcast(f32r)

    with tile.TileContext(nc) as tc, nc.allow_low_precision(
        reason="bf16/fp32r rounding on PE-matmul operands is intentional"
    ):
        with tc.tile_pool(name="persist", bufs=1) as pp:
            # persistent SBUF tensors (loaded on the scalar DMA queue,
            # which is idle at kernel start; sync carries h + wqa)
            wkvbk_sb = pp.tile([P, NKV, HLOC * DN], bf16, name="wkvbk")
            wkvbv_sb = pp.tile([P, NKV, HLOC * DV], bf16, name="wkvbv")
            trimask_sb = pp.tile([P, P], f32, name="trimask")
            cos_sb = pp.tile([P, TLOC], f32, name="cosL")
            nc.scalar.dma_start(out=cos_sb, in_=cosL_d[:, :])
            sin_sb = pp.tile([P, TLOC], f32, name="sinL")
            nc.scalar.dma_start(out=sin_sb, in_=sinL_d[:, :])

            ones_f = pp.tile([P, P], f32, name="ones_f")
            nc.vector.memset(ones_f, 1.0)
            ones_sb = pp.tile([P, 1], f32r, name="ones")
            nc.vector.tensor_copy(ones_sb, ones_f[:, 0:1])
            col_ones = pp.tile([1, P], f32r, name="col_ones")
            nc.vector.tensor_copy(col_ones, ones_f[0:1, :])
            eps_sb = pp.tile([1, 1], f32, name="eps")
            nc.vector.memset(eps_sb, EPS)

            # attention operand tensors (filled after the collectives)
            qTn = [pp.tile([P, T], f32r, name=f"qTn{h}") for h in range(HLOC)]
            qpeT2 = pp.tile([P, T], f32r, name="qpeT2")   # h0 rope rows 0:64, h1 64:128
            KT = [pp.tile([P, T], f32r, name=f"KT{h}") for h in range(HLOC)]
            kpe2 = [pp.tile([P, T], f32, name=f"kpe2{h}") for h in range(HLOC)]
            nc.vector.memset(kpe2[0][DR:P, :], 0.0)
            nc.vector.memset(kpe2[1][0:DR, :], 0.0)
            kvc_g = pp.tile([P, NKV, T], bf16, name="kvc_g")
            V_sb = [pp.tile([P, HLOC * DV], f32r, name=f"v{i}") for i in range(NKB)]

            # ---------------- Stage A: local projections ----------------
            with (
                tc.tile_pool(name="aloc", bufs=1) as ap_,
                tc.tile_pool(name="astream", bufs=2) as sp_,
                tc.tile_pool(name="asmall", bufs=1) as smp,
                tc.tile_pool(name="aps", bufs=2, space="PSUM") as s1ps,
                tc.tile_pool(name="qbps", bufs=4, space="PSUM") as qbps,
                tc.tile_pool(name="ssqps", bufs=1, space="PSUM") as ssqps,
            ):
                h_sb = ap_.tile([P, KH, TLOC], bf16, name="hmy")
                nc.sync.dma_start(out=h_sb, in_=hT_d[:, :, :])

                # ---- kv stage-1 first (feeds the AllGather ASAP) ----
                ssq_kv = ssqps.tile([1, TLOC], f32, name="ssq", bufs=2)
                kvraw = []
                kva_ps = []
                for m in range(NKV + 2):
                    wk_sb = sp_.tile([P, KH, P], bf16, name="wkstream")
                    nc.scalar.dma_start(out=wk_sb, in_=wkva_d[:, m, :, :])
                    ps = s1ps.tile([P, TLOC], f32, name="s1")
                    for k in range(KH):
                        nc.tensor.matmul(
                            ps,
                            lhsT=wk_sb[:, k, :],
                            rhs=h_sb[:, k, :],
                            start=(k == 0),
                            stop=(k == KH - 1),
                        )
                    if m < NKV:
                        kt = ap_.tile([P, TLOC], f32r, name=f"kvraw{m}")
                        nc.vector.tensor_copy(kt, ps)
                        kvraw.append(kt)
                        sq = smp.tile([P, TLOC], f32r, name="sq", bufs=1)
                        nc.scalar.square(sq, ps)
                        nc.tensor.matmul(
                            ssq_kv,
                            lhsT=r32(ones_sb),
                            rhs=r32(sq),
                            start=(m == 0),
                            stop=(m == NKV - 1),
                        )
                    else:
                        kva_ps.append(ps)   # rope A2/B2 read straight from PSUM

                # rkv = rsqrt(mean+eps), broadcast via ones-matmul
                rkv = smp.tile([1, TLOC], f32r, name="rkv")
                nc.scalar.activation(
                    rkv, ssq_kv, func=AF.Sqrt, bias=eps_sb, scale=1.0 / KVL
                )
                nc.vector.reciprocal(rkv, rkv)
                rkvb_ps = s1ps.tile([P, TLOC], f32, name="s1")
                nc.tensor.matmul(rkvb_ps, lhsT=col_ones, rhs=rkv, start=True, stop=True)
                rkv_b = smp.tile([P, TLOC], f32, name="rkvb")
                nc.vector.tensor_copy(rkv_b, rkvb_ps)

                # k_pe rope: (A*cos + B*sin); rows 0:64 are the head-shared k_pe
                t1 = smp.tile([P, TLOC], f32, name="ropet1")
                t2 = smp.tile([P, TLOC], f32, name="ropet2")
                nc.vector.tensor_mul(t1, kva_ps[0], cos_sb)
                nc.vector.tensor_mul(t2, kva_ps[1], sin_sb)
                nc.vector.tensor_add(t1, t1, t2)
                kpe_bf = smp.tile([DR, TLOC], bf16, name="kpebf")
                nc.vector.tensor_copy(kpe_bf, t1[0:DR, :])
                nc.scalar.dma_start(out=kv_snd[KVL : KVL + DR, :], in_=kpe_bf)

                # normalize latent, send
                for m in range(NKV):
                    kvn = ap_.tile([P, TLOC], bf16, name=f"kvn{m}")
                    nc.vector.tensor_mul(kvn, kvraw[m], rkv_b)
                    nc.scalar.dma_start(out=kv_snd[m * P : (m + 1) * P, :], in_=kvn)
                nc.gpsimd.collective_compute(
                    "AllGather",
                    mybir.AluOpType.bypass,
                    replica_groups=RG,
                    ins=[kv_snd[:, :]],
                    outs=[kv_rcv[:, :, :]],
                )
                # small weights needed from the kv_b phase on (~90us in)
                nc.scalar.dma_start(out=wkvbk_sb, in_=wkvbk_d[:, :, :])
                nc.scalar.dma_start(out=wkvbv_sb, in_=wkvbv_d[:, :, :])
                nc.scalar.dma_start(out=trimask_sb, in_=trimask_d[:, :])

                # ---- q stage-1 ----
                ssq_q = ssqps.tile([1, TLOC], f32, name="ssq", bufs=2)
                qcraw = []
                for m in range(NKQ):
                    wq_sb = sp_.tile([P, KH, P], bf16, name="wqstream")
                    nc.sync.dma_start(out=wq_sb, in_=wqa_d[:, m, :, :])
                    ps = s1ps.tile([P, TLOC], f32, name="s1")
                    for k in range(KH):
                        nc.tensor.matmul(
                            ps,
                            lhsT=wq_sb[:, k, :],
                            rhs=h_sb[:, k, :],
                            start=(k == 0),
                            stop=(k == KH - 1),
                        )
                    qt = ap_.tile([P, TLOC], f32r, name=f"qcraw{m}")
                    nc.vector.tensor_copy(qt, ps)
                    qcraw.append(qt)
                    sq = smp.tile([P, TLOC], f32r, name="sq", bufs=1)
                    nc.scalar.square(sq, ps)
                    nc.tensor.matmul(
                        ssq_q,
                        lhsT=r32(ones_sb),
                        rhs=r32(sq),
                        start=(m == 0),
                        stop=(m == NKQ - 1),
                    )

                rq = smp.tile([1, TLOC], f32r, name="rq")
                nc.scalar.activation(
                    rq, ssq_q, func=AF.Sqrt, bias=eps_sb, scale=1.0 / QL
                )
                nc.vector.reciprocal(rq, rq)
                rqb_ps = s1ps.tile([P, TLOC], f32, name="s1")
                nc.tensor.matmul(rqb_ps, lhsT=col_ones, rhs=rq, start=True, stop=True)
                rq_b = smp.tile([P, TLOC], f32, name="rqb")
                nc.vector.tensor_copy(rq_b, rqb_ps)
                qcn = []
                for m in range(NKQ):
                    qn = ap_.tile([P, TLOC], bf16, name=f"qcn{m}")
                    nc.vector.tensor_mul(qn, qcraw[m], rq_b)
                    qcn.append(qn)

                # ---- q_b for ALL heads on my tokens; pack A2A shards ----
                for sh in range(NC_):
                    wqb_sb = sp_.tile([P, 4, NKQ, P], bf16, name="wqbstream", bufs=3)
                    eng = nc.sync if sh % 2 == 0 else nc.scalar
                    eng.dma_start(out=wqb_sb, in_=wqb_d[:, sh, :, :, :])
                    ups = []
                    for mo in range(4):
                        pool = qbps if mo < 2 else s1ps
                        nm = "up" if mo < 2 else "s1"
                        ps = pool.tile([P, TLOC], f32, name=nm)
                        for k in range(NKQ):
                            nc.tensor.matmul(
                                ps,
                                lhsT=wqb_sb[:, mo, k, :],
                                rhs=qcn[k],
                                start=(k == 0),
                                stop=(k == NKQ - 1),
                            )
                        ups.append(ps)
                    for u in range(2):
                        st = smp.tile([P, TLOC], bf16, name="stn", bufs=2)
                        nc.vector.tensor_copy(st, ups[u])
                        nc.sync.dma_start(out=a_snd[sh, u, :, :], in_=st)
                    t3 = smp.tile([P, TLOC], f32, name="ropeq1", bufs=2)
                    t4 = smp.tile([P, TLOC], f32, name="ropeq2", bufs=2)
                    nc.vector.tensor_mul(t3, ups[2], cos_sb)
                    nc.vector.tensor_mul(t4, ups[3], sin_sb)
                    stp = smp.tile([P, TLOC], bf16, name="stp", bufs=2)
                    nc.vector.tensor_add(stp, t3, t4)
                    nc.sync.dma_start(out=a_snd[sh, 2, :, :], in_=stp)
                nc.gpsimd.collective_compute(
                    "AllToAll",
                    mybir.AluOpType.bypass,
                    replica_groups=RG,
                    ins=[a_snd[:, :, :, :]],
                    outs=[a_rcv[:, :, :, :]],
                )

                # ---- consume the gathered kv: kv_b up-projection ----
                for k in range(NKV):
                    nc.sync.dma_start(
                        out=kvc_g[:, k, :].rearrange("p (r t) -> p r t", t=TLOC),
                        in_=kv_rcv[:, k * P : (k + 1) * P, :].rearrange(
                            "r p t -> p r t"
                        ),
                    )
                # k_pe duplicated into both head slots (cast bf16->f32)
                nc.gpsimd.dma_start(
                    out=kpe2[0][0:DR, :].rearrange("p (r t) -> p r t", t=TLOC),
                    in_=kv_rcv[:, KVL:, :].rearrange("r p t -> p r t"),
                )
                nc.gpsimd.dma_start(
                    out=kpe2[1][DR:P, :].rearrange("p (r t) -> p r t", t=TLOC),
                    in_=kv_rcv[:, KVL:, :].rearrange("r p t -> p r t"),
                )
                # K^T per head over full T
                for mo in range(HLOC):
                    for cc in range(T // TLOC):
                        ps = qbps.tile([P, TLOC], f32, name="up")
                        for k in range(NKV):
                            nc.tensor.matmul(
                                ps,
                                lhsT=wkvbk_sb[:, k, mo * P : (mo + 1) * P],
                                rhs=kvc_g[:, k, cc * TLOC : (cc + 1) * TLOC],
                                start=(k == 0),
                                stop=(k == NKV - 1),
                            )
                        nc.vector.tensor_copy(KT[mo][:, cc * TLOC : (cc + 1) * TLOC], ps)
                # V in natural orientation [t, 2*dv]
                for tt in range(NKB):
                    ps = qbps.tile([P, HLOC * DV], f32, name="up")
                    for k in range(NKV):
                        nc.tensor.matmul(
                            ps,
                            lhsT=kvc_g[:, k, tt * P : (tt + 1) * P],
                            rhs=wkvbv_sb[:, k, :],
                            start=(k == 0),
                            stop=(k == NKV - 1),
                        )
                    nc.vector.tensor_copy(V_sb[tt], ps)

                # ---- consume the A2A'd q (cast bf16->f32r) ----
                for hh in range(HLOC):
                    nc.gpsimd.dma_start(
                        out=qTn[hh][:, :].rearrange("p (r t) -> p r t", t=TLOC),
                        in_=a_rcv[:, hh, :, :].rearrange("r p t -> p r t"),
                    )
                nc.gpsimd.dma_start(
                    out=qpeT2[:, :].rearrange("p (r t) -> p r t", t=TLOC),
                    in_=a_rcv[:, 2, :, :].rearrange("r p t -> p r t"),
                )

            # ---------------- Stage B: attention ----------------
            with (
                tc.tile_pool(name="bpt", bufs=4) as ptp,
                tc.tile_pool(name="bsmall", bufs=3) as bsm,
                tc.tile_pool(name="sps", bufs=2, space="PSUM") as spsp,
                tc.tile_pool(name="otps", bufs=2, space="PSUM") as otpsp,
                tc.tile_pool(name="lps", bufs=2, space="PSUM") as lpsp,
            ):
                OT_sb = [
                    [ptp.tile([P, QC], f32r, name=f"ot{h}_{j}", bufs=1) for j in range(NQC)]
                    for h in range(HLOC)
                ]
                # o_proj weights, needed only at the tail (gpsimd is idle here)
                wo_sb = ptp.tile([P, HLOC, T], f32r, name="wo", bufs=1)
                nc.gpsimd.dma_start(out=wo_sb, in_=wo_d[:, :, :])

                def flush_norm(pend):
                    p_ot, p_l, p_h, p_j = pend
                    recl = bsm.tile([1, QC], f32r, name="recl")
                    nc.vector.reciprocal(recl, p_l)
                    lb_ps = spsp.tile([P, 2 * QC], f32, name="sps2")[:, :QC]
                    nc.tensor.matmul(lb_ps, lhsT=col_ones, rhs=recl, start=True, stop=True)
                    lb = bsm.tile([P, QC], f32, name="lb")
                    nc.scalar.copy(lb, lb_ps)
                    nc.vector.tensor_mul(OT_sb[p_h][p_j], p_ot, lb)

                def emit_pv(job):
                    """Mask + P@V + denominator matmuls for an exp'd pair.
                    Returns the (h, j) accumulators when this pair completes
                    them (so normalization can be scheduled)."""
                    pt, kp, nkb, ot_ps, l_ps, jh, jj = job
                    for u in range(2):
                        ki = kp + u
                        diag = (ki // 4 == jj)
                        cs = (ki % 4) * P if diag else 0
                        if diag:
                            nc.gpsimd.tensor_mul(
                                pt[:, u * QC + cs : u * QC + cs + P],
                                pt[:, u * QC + cs : u * QC + cs + P],
                                trimask_sb,
                            )
                        nc.tensor.matmul(
                            ot_ps[:, cs:],
                            lhsT=r32(V_sb[ki][:, jh * DV : (jh + 1) * DV]),
                            rhs=r32(pt[:, u * QC + cs : (u + 1) * QC]),
                            start=(ki == 0),
                            stop=(ki == nkb - 1),
                        )
                        nc.tensor.matmul(
                            l_ps[:, cs:],
                            lhsT=r32(ones_sb),
                            rhs=r32(pt[:, u * QC + cs : (u + 1) * QC]),
                            start=(ki == 0),
                            stop=(ki == nkb - 1),
                        )
                    if kp == nkb - 2:
                        return (ot_ps, l_ps, jh, jj)
                    return None

                # software-pipelined: pair k+1's scores+exp are emitted BEFORE
                # pair k's PV/l so the in-order PE queue never waits on the ACT
                # exp (bubbles there reset the PE frequency ramp)
                prev = None       # exp'd pair awaiting PV/l
                pendflush = None  # completed (h,j) awaiting normalization
                for h in range(HLOC):
                    for j in range(NQC):
                        ot_ps = otpsp.tile([P, QC], f32, name="otps")
                        l_ps = lpsp.tile([1, QC], f32, name="lps")
                        nkb = 4 * (j + 1)
                        qcol0 = j * QC
                        for kp in range(0, nkb, 2):
                            # two k-blocks share one PSUM pair and ONE wide exp
                            s2 = spsp.tile([P, 2 * QC], f32, name="sps2")
                            for u in range(2):
                                ki = kp + u
                                nc.tensor.matmul(
                                    s2[:, u * QC : (u + 1) * QC],
                                    lhsT=r32(KT[h][:, ki * P : (ki + 1) * P]),
                                    rhs=r32(qTn[h][:, qcol0 : qcol0 + QC]),
                                    start=True,
                                    stop=False,
                                )
                                nc.tensor.matmul(
                                    s2[:, u * QC : (u + 1) * QC],
                                    lhsT=r32(kpe2[h][:, ki * P : (ki + 1) * P]),
                                    rhs=r32(qpeT2[:, qcol0 : qcol0 + QC]),
                                    start=False,
                                    stop=True,
                                )
                            pt = ptp.tile([P, 2 * QC], f32r, name="pt")
                            nc.scalar.activation(pt, s2, func=AF.Exp, scale=SCALE)
                            if prev is not None:
                                done = emit_pv(prev)
                                if pendflush is not None:
                                    flush_norm(pendflush)
                                    pendflush = None
                                if done is not None:
                                    pendflush = done
                            prev = (pt, kp, nkb, ot_ps, l_ps, h, j)
                done = emit_pv(prev)
                if pendflush is not None:
                    flush_norm(pendflush)
                flush_norm(done)

                # ---------------- o_proj (partial y, summed on host) ----------------
                for tt in range(T // P):
                    j, sub = tt // 4, (tt % 4) * P
                    for n in range(HID // QC):
                        y_ps = spsp.tile([P, 2 * QC], f32, name="sps2")[:, :QC]
                        for h in range(HLOC):
                            nc.tensor.matmul(
                                y_ps,
                                lhsT=r32(OT_sb[h][j][:, sub : sub + P]),
                                rhs=r32(wo_sb[:, h, n * QC : (n + 1) * QC]),
                                start=(h == 0),
                                stop=(h == HLOC - 1),
                            )
                        y_sb = ptp.tile([P, QC], bf16, name="ysb")
                        nc.vector.tensor_copy(y_sb, y_ps)
                        eng = nc.sync if tt % 2 == 0 else nc.scalar
                        eng.dma_start(
                            out=y_d[tt * P : (tt + 1) * P, n * QC : (n + 1) * QC],
                            in_=y_sb,
                        )
    _split_multi_waits(nc, mybir)
    return nc


def _host_prep(inputs):
    import ml_dtypes

    bf = ml_dtypes.bfloat16
    hs = np.ascontiguousarray(np.asarray(inputs["hidden_states"], np.float32))
    pos = np.asarray(inputs["positions"], np.int32)
    w_qa = np.asarray(inputs["w_qa"], np.float32)
    q_ln = np.asarray(inputs["q_a_ln_w"], np.float32)
    w_qb = np.asarray(inputs["w_qb"], np.float32)
    w_kva = np.asarray(inputs["w_kva"], np.float32)
    kv_ln = np.asarray(inputs["kv_a_ln_w"], np.float32)
    w_kvb = np.asarray(inputs["w_kvb"], np.float32)
    w_o = np.asarray(inputs["w_o"], np.float32)

    # replicated stage-1 weights, baseline layout
    wqa_b = np.ascontiguousarray(
        w_qa.reshape(HID // P, P, QL // P, P).transpose(1, 2, 0, 3)
    ).astype(bf)

    # rope tables (neox), rows duplicated for the 2-head-slot packing
    inv_freq = (1.0 / (10000.0 ** (np.arange(0, DR, 2, dtype=np.float32) / DR))).astype(
        np.float32
    )
    freqs = pos.astype(np.float32)[:, None] * inv_freq[None, :]
    emb = np.concatenate([freqs, freqs], axis=-1)  # [T, 64]
    cosT = np.ascontiguousarray(np.cos(emb).T.astype(np.float32))  # [64, T]
    sinT = np.ascontiguousarray(np.sin(emb).T.astype(np.float32))
    cos2 = np.ascontiguousarray(np.concatenate([cosT, cosT], axis=0))  # [128, T]
    sin2 = np.ascontiguousarray(np.concatenate([sinT, sinT], axis=0))

    def rot_cols(A):
        return np.concatenate([-A[:, DR // 2 :], A[:, : DR // 2]], axis=1)

    # kv a-projection augmented with duplicated rope A/B columns
    kva_lat = w_kva[:, :KVL]
    kva_rope = w_kva[:, KVL:]                       # [2048, 64]
    kva_ropeB = rot_cols(kva_rope)
    wkva_aug = np.concatenate(
        [kva_lat, kva_rope, kva_rope, kva_ropeB, kva_ropeB], axis=1
    )  # [2048, 768]
    wkva_b = np.ascontiguousarray(
        wkva_aug.reshape(HID // P, P, NKV + 2, P).transpose(1, 2, 0, 3)
    ).astype(bf)

    w_qb_f = (w_qb * q_ln[:, None]).reshape(QL, H, DQK)
    w_kvb_f = (w_kvb * kv_ln[:, None]).reshape(KVL, H, DN + DV)
    w_o_r = w_o.reshape(H, DV, HID)

    # q_b for ALL heads, shard-major: [p, shard, mo, k, j]
    shards = []
    for sh in range(NC_):
        e, o = 2 * sh, 2 * sh + 1
        nope_e = w_qb_f[:, e, :DN]
        nope_o = w_qb_f[:, o, :DN]
        peA2 = np.concatenate([w_qb_f[:, e, DN:], w_qb_f[:, o, DN:]], axis=1)
        peB2 = np.concatenate(
            [rot_cols(w_qb_f[:, e, DN:]), rot_cols(w_qb_f[:, o, DN:])], axis=1
        )
        shards.append(np.concatenate([nope_e, nope_o, peA2, peB2], axis=1))  # [QL,512]
    wqb_all = np.stack(shards, axis=1)  # [QL, 8, 512]
    wqb_dev = np.ascontiguousarray(
        wqb_all.reshape(NKQ, P, NC_, 4, P).transpose(1, 2, 3, 0, 4)
    ).astype(bf)

    trimask = np.triu(np.ones((P, P), dtype=np.float32))  # [k, q]: 1 iff q>=k

    per_core = []
    for i in range(NC_):
        hh = [HLOC * i + x for x in range(HLOC)]
        t0 = i * TLOC
        hT = np.ascontiguousarray(
            hs[t0 : t0 + TLOC].T.reshape(KH, P, TLOC).transpose(1, 0, 2)
        ).astype(bf)
        wkvbk = np.ascontiguousarray(
            np.concatenate([w_kvb_f[:, h, :DN] for h in hh], axis=1)
            .reshape(NKV, P, HLOC * DN)
            .transpose(1, 0, 2)
        ).astype(bf)
        wkvbv = np.ascontiguousarray(
            np.concatenate([w_kvb_f[:, h, DN:] for h in hh], axis=1)
            .reshape(NKV, P, HLOC * DV)
            .transpose(1, 0, 2)
        ).astype(bf)
        wo_i = np.ascontiguousarray(
            np.stack([w_o_r[h] for h in hh], axis=0).transpose(1, 0, 2)
        )  # [p, h, HID]
        per_core.append(
            dict(
                hT=hT,
                wqa=wqa_b,
                wkva=wkva_b,
                wqb=wqb_dev,
                wkvbk=wkvbk,
                wkvbv=wkvbv,
                wo=wo_i,
                cosL=np.ascontiguousarray(cos2[:, t0 : t0 + TLOC]),
                sinL=np.ascontiguousarray(sin2[:, t0 : t0 + TLOC]),
                trimask=trimask,
            )
        )
    return per_core


def kernel(**inputs):
    global LAST_RESULTS
    from concourse.bass_utils import run_bass_kernel_spmd

    if "nc" not in _CACHE:
        _CACHE["nc"] = _build_program()
    nc = _CACHE["nc"]

    in_maps = _host_prep(inputs)
    res = run_bass_kernel_spmd(nc, in_maps, core_ids=list(range(NC_)))
    LAST_RESULTS = res
    out = np.zeros((T, HID), dtype=np.float32)
    for r in res.results:
        out += np.asarray(r["y"], dtype=np.float32)
    return out
